# revision 9
# baseline (speedup 1.0000x reference)
"""AdaptiveWaveletTransform on 8 TRN2 NeuronCores.

Math: for each of 8 scales, out[b,s,t,f] = sum_l kern_s[l] * signal[b,t-l,f]
(causal full-conv truncated to t in [0,4096)), kern_s = linear-interp dilated
Morlet wavelet of length L_s = int(64*scale_s), followed by scale_weights
multiply and |x|>1e-4 sparsity masking.

Device mapping: depthwise time-conv == banded-Toeplitz matmul.  The 1024
sequences (16 batches x 64 features) are split into two halves of 512
(= matmul free dim N).  4 cores per half; core c owns time-tiles
{c, c+4, ..., c+28} (stride-4 interleave balances the causal-boundary
savings).  Per (owned-tile j, scale s) the conv accumulates
min(nk_s, 4j+4) [128x128] bf16 Toeplitz blocks into a PSUM bank.  All 8
cores run one SPMD graph; per-core differences live entirely in the data:
the signal shard is pre-shifted by c tile-slots on the host, with zeros
where the (boundary) tile index falls below 0.

j-outer loop order + signal DMA split into 8 chunks lets matmuls start
after ~0.5 MB of input lands.  Weights are k-major so the first chunk
covers every k<4 block.  Epilogue per tile: sq=Square(acc) on ScalarE,
mask=(sq>thr^2) and out=acc*mask on VectorE, DMA out fp32.
Host reassembles [16,8,4096,64].
"""

import os
import sys

import numpy as np
import ml_dtypes

import concourse.bass as bass
from concourse import bacc
import concourse.mybir as mybir
import concourse.tile as tile
from concourse.bass_utils import run_bass_kernel_spmd

# ---------------------------------------------------------------- constants
B, S, F = 16, 4096, 64
WAVELET_LEN = 64
N_SCALES = 8
THR = 1e-4
P = 128
NSEQ = 512            # sequences per half (8 batches x 64 features)
NT = S // P           # 32 time tiles
JT = 8                # owned time tiles per core
NSLOT = 32            # signal slots; slot s holds tile (s + c - 3) on core c
NCHUNK = 8            # signal DMA chunks of 4 slots

_scales = np.logspace(np.log10(1.0), np.log10(32.0), N_SCALES)
_Ls = [int(WAVELET_LEN * float(s)) for s in _scales]
_nks = [(L - 1 + 127) // 128 + 1 for L in _Ls]
# k-major block order so the first weight chunk covers all k<4 blocks
_border = sorted((k, s) for s in range(N_SCALES) for k in range(_nks[s]))
_bidx = {(s, k): i for i, (k, s) in enumerate(_border)}
NBLK = len(_border)   # 51
WCHUNK0 = sum(1 for (k, s) in _border if k < 4)  # 27

_bf16 = ml_dtypes.bfloat16

_GRAPH_CACHE = {}
LAST_EXEC_TIME_NS = None
PROFILE = True
PROFILE_DIR = None
PROFILE_ALL_CORES = False


def _build_graph():
    """Build the SPMD bass graph (identical on all 8 cores)."""
    nc = bacc.Bacc()
    sig_ext = nc.declare_dram_parameter(
        "sig", [P, NSLOT, NSEQ], mybir.dt.bfloat16, isOutput=False
    )
    wts_ext = nc.declare_dram_parameter(
        "wts", [P, NBLK, P], mybir.dt.bfloat16, isOutput=False
    )
    out_ext = nc.declare_dram_parameter(
        "out", [N_SCALES, JT, P, NSEQ], mybir.dt.bfloat16, isOutput=True
    )

    with tile.TileContext(nc) as tc:
        with (
            tc.tile_pool(name="const", bufs=1) as const_pool,
            tc.tile_pool(name="sig", bufs=NCHUNK) as sig_pool,
            tc.tile_pool(name="stage", bufs=4) as stage_pool,
            tc.tile_pool(name="psum", bufs=8, space="PSUM") as psum_pool,
        ):
            wts_sb = const_pool.tile([P, NBLK, P], mybir.dt.bfloat16)
            chunks = [
                sig_pool.tile([P, 4, NSEQ], mybir.dt.bfloat16,
                              tag="sigch", name=f"sigch_{m}")
                for m in range(NCHUNK)
            ]
            # k-major weight order: blocks [0, W0) are k=0, [0, WCHUNK0) are k<4.
            # Order transfers by first use: k=0 weights + sig chunk 0 gate the
            # first matmuls of j=0.
            W0 = N_SCALES  # k=0 group: one block per scale
            nc.sync.dma_start(wts_sb[:, :W0, :], wts_ext[:, :W0, :])
            nc.sync.dma_start(chunks[0][:], sig_ext[:, 0:4, :])
            nc.sync.dma_start(wts_sb[:, W0:WCHUNK0, :], wts_ext[:, W0:WCHUNK0, :])
            for m in range(1, NCHUNK):
                nc.sync.dma_start(chunks[m][:], sig_ext[:, 4 * m:4 * m + 4, :])
            nc.sync.dma_start(wts_sb[:, WCHUNK0:, :], wts_ext[:, WCHUNK0:, :])

            def rhs(slot):
                return chunks[slot // 4][:, slot % 4, :]

            for j in range(JT):
                acc = [
                    psum_pool.tile([P, NSEQ], mybir.dt.float32, tag="acc",
                                   name=f"acc_{j}_{s}")
                    for s in range(N_SCALES)
                ]
                for s_idx in range(N_SCALES):
                    nb = min(_nks[s_idx], 4 * j + 4)
                    for k in range(nb):
                        nc.tensor.matmul(
                            acc[s_idx][:],
                            lhsT=wts_sb[:, _bidx[(s_idx, k)], :],
                            rhs=rhs(3 + 4 * j - k),
                            start=(k == 0),
                            stop=(k == nb - 1),
                        )
                    sq = stage_pool.tile([P, NSEQ], mybir.dt.bfloat16, tag="sq")
                    outt = stage_pool.tile([P, NSEQ], mybir.dt.bfloat16, tag="outt")
                    nc.scalar.square(sq[:], acc[s_idx][:])
                    nc.vector.tensor_scalar(
                        sq[:], sq[:], THR * THR, None, mybir.AluOpType.is_gt
                    )
                    nc.vector.tensor_tensor(
                        outt[:], acc[s_idx][:], sq[:], mybir.AluOpType.mult
                    )
                    nc.sync.dma_start(out_ext[s_idx, j], outt[:])
    nc.compile()
    return nc


def _host_weights(mother_wavelets, scale_weights):
    """Toeplitz blocks [P, NBLK, P] bf16: wts[jj, bidx[s,k], i] = kern_s[128k+i-jj]."""
    wts = np.zeros((P, NBLK, P), dtype=np.float32)
    ii = np.arange(P)[None, :]
    jj = np.arange(P)[:, None]
    for s_idx in range(N_SCALES):
        scale = float(_scales[s_idx])
        L = _Ls[s_idx]
        xq = np.linspace(0.0, float(WAVELET_LEN - 1), L)
        grid = np.arange(WAVELET_LEN, dtype=np.float64)
        kern = np.interp(xq, grid, mother_wavelets[s_idx].astype(np.float64))
        kern = kern / np.sqrt(scale) * float(scale_weights[s_idx])
        kern = kern.astype(np.float32)
        kpad = np.zeros(128 * _nks[s_idx] + 256, dtype=np.float32)
        kpad[:L] = kern
        for k in range(_nks[s_idx]):
            idx = 128 * k + ii - jj
            blk = np.where((idx >= 0) & (idx < L), kpad[np.clip(idx, 0, len(kpad) - 1)], 0.0)
            wts[:, _bidx[(s_idx, k)], :] = blk
    return wts.astype(_bf16)


def _ntff_hook():
    """ctypes NTFF profile start/stop via the axon PJRT plugin, or None."""
    try:
        import ctypes
        so = "/opt/axon/libaxon_pjrt.so"
        if not os.path.exists(so):
            return None
        lib = ctypes.CDLL(so)
        if not hasattr(lib, "axon_start_nrt_profile"):
            return None
        lib.axon_start_nrt_profile.argtypes = [
            ctypes.POINTER(ctypes.c_int64), ctypes.c_size_t]
        lib.axon_start_nrt_profile.restype = ctypes.c_int64
        lib.axon_stop_nrt_profile.argtypes = [ctypes.c_char_p]
        lib.axon_stop_nrt_profile.restype = ctypes.c_int64
        return lib
    except Exception:
        return None


def _parse_exec_time(outdir, nc, cores=(0,)):
    """NTFF -> neuron-profile json -> exec_time_ns (max over cores)."""
    from concourse._compat import FishPath
    import gauge.profiler as gp
    from gauge import trn_perfetto

    prof = gp.Profile(profile_path=FishPath(outdir), kernel_dev_mode=True,
                      profile_on_exit=False, bass_kernel=nc.m,
                      offline_processing=True, fname="*_body*")
    prof.convert_ntffs_to_json(tuple(cores))
    times = []
    for c in cores:
        jp = prof.json_path(c)
        if not jp.is_file():
            continue
        conv = trn_perfetto.TrnPerfettoConv(kernel_dev_mode=True, bass_kernel=nc.m)
        conv.load_json(jp.path)
        conv.process()
        if conv.last_useful_time is not None and conv.first_useful_time is not None:
            times.append(conv.last_useful_time - conv.first_useful_time)
    return max(times) if times else None


def kernel(signal, mother_wavelets, scale_weights):
    global LAST_EXEC_TIME_NS, PROFILE_DIR
    signal = np.asarray(signal, dtype=np.float32)
    mother_wavelets = np.asarray(mother_wavelets, dtype=np.float32)
    scale_weights = np.asarray(scale_weights, dtype=np.float32)
    assert signal.shape == (B, S, F)

    if "nc" not in _GRAPH_CACHE:
        _GRAPH_CACHE["nc"] = _build_graph()
    nc = _GRAPH_CACHE["nc"]

    wts = _host_weights(mother_wavelets, scale_weights)

    # per-half time-major signal [S, 512] -> tiles [32, 128, 512] bf16
    in_maps = []
    for h in range(2):
        half = signal[h * 8:(h + 1) * 8]                      # [8, S, F]
        half = half.transpose(1, 0, 2).reshape(S, NSEQ)       # [S, 512]
        tiles = half.astype(_bf16).reshape(NT, P, NSEQ)       # [32, 128, 512]
        for c in range(4):
            shard = np.zeros((P, NSLOT, NSEQ), dtype=_bf16)
            # slot s holds signal tile (s + c - 3); zeros below tile 0
            shard[:, 3 - c:, :] = tiles[:NT - 3 + c].transpose(1, 0, 2)
            in_maps.append({"sig": shard, "wts": wts})

    lib = _ntff_hook() if PROFILE else None
    if lib is not None:
        try:
            import tempfile
            import jax
            jax.devices()
            PROFILE_DIR = tempfile.mkdtemp(prefix="awt_ntff_")
            rc = lib.axon_start_nrt_profile(None, 0)
            if rc != 0:
                lib = None
        except Exception:
            lib = None

    res = run_bass_kernel_spmd(nc, in_maps, core_ids=list(range(8)))

    LAST_EXEC_TIME_NS = res.exec_time_ns
    if lib is not None:
        try:
            n = lib.axon_stop_nrt_profile(PROFILE_DIR.encode())
            if n > 0:
                cores = range(8) if PROFILE_ALL_CORES else (0,)
                t = _parse_exec_time(PROFILE_DIR, nc, cores)
                if t is not None:
                    LAST_EXEC_TIME_NS = t
        except Exception as e:
            print(f"NTFF profiling failed: {e}", file=sys.stderr)
    if LAST_EXEC_TIME_NS is not None:
        print(f"HW exec time: {LAST_EXEC_TIME_NS} ns")

    out = np.empty((B, N_SCALES, S, F), dtype=np.float32)
    for i in range(8):
        h, c = divmod(i, 4)
        arr = res.results[i]["out"].astype(np.float32).reshape(N_SCALES, JT, P, 8, F)
        for j in range(JT):
            m = 4 * j + c
            # arr[:, j] = [scale, 128, 8, F] -> [8, scale, 128, F]
            out[h * 8:(h + 1) * 8, :, m * P:(m + 1) * P, :] = arr[:, j].transpose(2, 0, 1, 3)
    return out
